# revision 36
# baseline (speedup 1.0000x reference)
"""Trainium2 Bass kernel for a hard-triplet margin-ranking loss.

Sharding: data-parallel over anchor rows. 8 cores x 512 rows each. Rows in
the first half of the batch mine over columns [2048:4096], rows in the second
half over [0:2048], so each core needs only its 512x2048 slice of the
distance matrix. Per core:

  1. Features arrive as fp16 (host-converted; 5e-4 relative quantization,
     far inside the 2e-2 tolerance) in five [128, 4x256] group tiles on two
     parallel DGE queues; both ACT function-table sets are warmed in the
     DMA shadow.
  2. Row norms per group: one wide ACT Square into fp16 scratch, four
     4x-fast-mode DVE tensor_scalar add-accumulates (one per 256-wide
     tile), ACT sqrt(4*sq) and DVE reciprocal give inv = 1/(2*||x||) (the
     reference's +1e-6 shifts the result by <1e-7 and is dropped).
     Per-group chains pipeline against later groups' DMAs/squares.
  3. Normalize via DVE tensor_scalar in fp16 (4x fast mode; BOTH sides
     scaled by 0.5*inv so pm = 0.25*<xn_i,xn_j>, dist^2 = 2 - 8*pm),
     PE-transpose per group (fp16: 1 cycle/row vs 4 for fp32), evacuate
     PSUM per group with one ACT copy into fp16 xT tiles.
  4. Same-class mask built once per row tile as mq = (t_o == t_b) in fp16
     (tensor_scalar is_equal, 4x fast mode; opposite-half targets arrive
     host-pre-broadcast to [128, 2048] fp16, replacing an on-chip
     broadcast with DMA).
  5. pm = xbT.T @ xoT on PE (fp16, fp32 PSUM accumulate, K=256) PLUS a
     third K-chunk (-I).T @ mq_r that subtracts the same-class mask inside
     PSUM on the otherwise idle PE -- no separate DVE mask pass.  Matched
     columns land in [-1.25, -0.75], unmatched in [-0.25, 0.25].  Each row
     tile is two independent [128,1024] PSUM halves so the lo half
     (opposite groups 1-2) reduces while groups 3-4 are still in flight,
     and freeing the lo buffer early unblocks the next row-tile pair.
  6. Per half, ONE pass evacuates PSUM to fp16 AND row-max/min
     accumulates: a tensor_scalar with accum_out reduces with op1 (the
     interp's TENSOR_REDUCE_OPS[op1]), so the f32 PSUM read doubles as the
     max (or the ACT engine does a plain copy for the halves that land in
     its idle window, leaving DVE a 4x-fast-mode fp16 accum-reduce); the
     second reduction re-reads the fp16 copy at the 4x fast mode.  Tiny
     TT max/min ops merge the halves.  Hardest positive = row MIN (masked
     down), hardest negative = row MAX.
  7. dist_ap = sqrt(relu(-8*mn - 6)) (exact 0 when a row has no
     positives), dist_an = sqrt(relu(-8*mx + 2) + eps), overridden to 1.0
     via copy_predicated where mx < -0.5 (only when every column is
     same-class); row loss = relu(dist_ap - dist_an + margin);
     ones-matmul row-sum, run as two halves overlapping the main loop.
  8. Host sums the 8 per-core partial sums / 4096.
"""

import numpy as np

N, D = 4096, 256
HALF = N // 2
NCORES = 8
RPC = N // NCORES  # 512 anchor rows per core
RT = RPC // 128    # 4 anchor row tiles
OT = HALF // 128   # 16 opposite-half tiles
NT = RT + OT       # 20 input tiles
NG = NT // 4       # 5 groups of 4 tiles
MARGIN = 0.3
EPS = 1e-6
S = 0.125          # anchor pre-scale: pm = -2*S*dot = -0.25*dot

_CACHE = {}


def _build():
    from contextlib import ExitStack

    import concourse.bacc as bacc
    import concourse.bass as bass
    import concourse.tile as tile
    from concourse import masks, mybir

    f32 = mybir.dt.float32
    f16 = mybir.dt.float16
    Alu = mybir.AluOpType
    Act = mybir.ActivationFunctionType
    AxX = mybir.AxisListType.X
    ts = bass.ts

    nc = bacc.Bacc(
        "TRN2",
        target_bir_lowering=False,
        debug=False,
        enable_asserts=True,
        num_devices=NCORES,
    )
    xb = nc.dram_tensor("xb", [128, RT * D], f16, kind="ExternalInput").ap()
    xo = nc.dram_tensor("xo", [128, OT * D], f16, kind="ExternalInput").ap()
    tb = nc.dram_tensor("tb", [128, RT], f32, kind="ExternalInput").ap()
    to = nc.dram_tensor("to", [128, HALF], f16, kind="ExternalInput").ap()
    out = nc.dram_tensor("out", [1, 1], f32, kind="ExternalOutput").ap()

    with tile.TileContext(nc) as tc, ExitStack() as ctx:
        const = ctx.enter_context(tc.tile_pool(name="const", bufs=1))
        xin = ctx.enter_context(tc.tile_pool(name="xin", bufs=1))
        xt = ctx.enter_context(tc.tile_pool(name="xt", bufs=1))
        stat = ctx.enter_context(tc.tile_pool(name="stat", bufs=1))
        scr = ctx.enter_context(tc.tile_pool(name="scr", bufs=4))
        wscr = ctx.enter_context(tc.tile_pool(name="wscr", bufs=3))
        mqp = ctx.enter_context(tc.tile_pool(name="mqp", bufs=4))
        tree = ctx.enter_context(tc.tile_pool(name="tree", bufs=4))
        psum = ctx.enter_context(tc.tile_pool(name="psum", bufs=4, space="PSUM"))

        # Feature tiles in 5 groups of 4: group 0 = anchors, 1..4 =
        # opposite.  Anchors + targets on the SP DGE queue, opposite-half
        # groups on the Pool DGE queue so the first group isn't stuck
        # behind 1.5MB of queue-serial transfers.
        xg = []
        for g in range(NG):
            gt = xin.tile([128, 4 * D], f16, tag=f"xg{g}")
            if g == 0:
                nc.sync.dma_start(gt[:], xb[:])
            else:
                nc.gpsimd.dma_start(gt[:], xo[:, (g - 1) * 4 * D : g * 4 * D])
            xg.append(gt)

        # Targets: opposite-half row pre-broadcast on host to [128, HALF];
        # per-row targets as one [128, RT] per-partition scalar bank.
        tbt = const.tile([128, RT], f32, tag="tbt")
        nc.sync.dma_start(tbt[:], tb[:])
        tob = const.tile([128, HALF], f16, tag="tob")
        nc.sync.dma_start(tob[:], to[:])

        ident = const.tile([128, 128], f16, tag="ident")
        masks.make_identity(nc, ident[:])
        # jneg = -I: the mask K-chunk adds -1 to same-class columns so the
        # hardest positive is the row MIN and the hardest negative the MAX.
        jneg = const.tile([128, 128], f16, tag="jneg")
        nc.gpsimd.memset(jneg[:], 0.0)
        nc.gpsimd.affine_select(
            out=jneg[:], in_=jneg[:], compare_op=Alu.not_equal,
            fill=-1.0, base=0, pattern=[[-1, 128]], channel_multiplier=1,
        )
        ones = const.tile([128, 1], f32, tag="ones")
        nc.vector.memset(ones[:], 1.0)

        # Warm both ACT function-table sets (Square/Copy and Sqrt/Relu)
        # during the DMA shadow so no LoadActFuncSet lands mid-pipeline.
        warm = const.tile([128, 1], f32, tag="warm")
        nc.scalar.activation(warm[:], ones[:], Act.Square)
        nc.scalar.activation(warm[:], ones[:], Act.Sqrt)
        nc.scalar.activation(warm[:], ones[:], Act.Relu)

        # Per group: wide square -> per-tile sum -> sqrt(4*sq) -> recip ->
        # normalize -> transpose -> evac.  Both sides scale by 0.5*inv so
        # pm = 0.25*<xn_i,xn_j> and the mask enters with -1 (jneg below).
        # Squares for the first groups are issued ahead of the chains so
        # ACT interleaves squares with evacuations without idling.
        xT = [None] * NG

        def issue_square(g):
            s2 = wscr.tile([128, 1024], f16, tag="s2")
            nc.scalar.activation(s2[:], xg[g][:], Act.Square)
            sqg = stat.tile([128, 4], f32, tag=f"sq{g}")
            for i in range(4):
                sd = scr.tile([128, D], f16, tag="sq_dummy")
                nc.vector.tensor_scalar(
                    sd[:], s2[:, ts(i, D)], 1.0, None,
                    op0=Alu.mult, op1=Alu.add, accum_out=sqg[:, i : i + 1],
                )
            return sqg

        def issue_chain(g, sqg):
            # inv = 1/(2*nrm) via sqrt(4*sq); the +eps of the reference
            # changes the result by <1e-7 and is dropped.
            nrm2 = stat.tile([128, 4], f32, tag=f"nrm{g}")
            nc.scalar.activation(nrm2[:], sqg[:], Act.Sqrt, scale=4.0)
            inv = stat.tile([128, 4], f32, tag=f"inv{g}")
            nc.vector.reciprocal(inv[:], nrm2[:])
            gt = xt.tile([128, 1024], f16, tag=f"xT{g}")
            pt = psum.tile([128, 1024], f16, tag="ps")
            for i in range(4):
                xn = scr.tile([128, D], f16, tag="xn")
                nc.vector.tensor_scalar_mul(
                    xn[:], xg[g][:, ts(i, D)], inv[:, i : i + 1]
                )
                for c in range(2):
                    nc.tensor.transpose(
                        pt[:, ts(c * 4 + i, 128)], xn[:, ts(c, 128)], ident[:]
                    )
            nc.scalar.copy(gt[:], pt[:])
            xT[g] = gt

        sqs = [issue_square(g) for g in range(3)]
        issue_chain(0, sqs[0])
        sqs.append(issue_square(3))
        issue_chain(1, sqs[1])
        sqs.append(issue_square(4))
        for g in range(2, NG):
            issue_chain(g, sqs[g])

        # Same-class masks, one per anchor row tile (4x fast mode).
        mq = []
        for r in range(RT):
            m = mqp.tile([128, HALF], f16, tag=f"mq{r}")
            nc.vector.tensor_scalar(
                m[:], tob[:], tbt[:, r : r + 1], None, op0=Alu.is_equal
            )
            mq.append(m)

        # Epilogue tiles + bias constants (epilogue runs in two halves so
        # tiles 0-1 decode while tiles 2-3 are still reducing).
        bm6 = const.tile([128, 1], f32, tag="bm6")
        nc.vector.memset(bm6[:], -6.0)
        bp2 = const.tile([128, 1], f32, tag="bp2")
        nc.vector.memset(bp2[:], 2.0)
        beps = const.tile([128, 1], f32, tag="beps")
        nc.vector.memset(beps[:], EPS)
        mx = stat.tile([128, RT], f32, tag="mx")
        mn = stat.tile([128, RT], f32, tag="mn")
        u = stat.tile([128, RT], f32, tag="u")
        dap = stat.tile([128, RT], f32, tag="dap")
        v = stat.tile([128, RT], f32, tag="v")
        sv = stat.tile([128, RT], f32, tag="sv")
        e = stat.tile([128, RT], mybir.dt.int32, tag="e")
        df = stat.tile([128, RT], f32, tag="df")
        lrow = stat.tile([128, RT], f32, tag="lrow")
        ones4 = const.tile([128, RT], f32, tag="ones4")
        nc.vector.memset(ones4[:], 1.0)

        def epilogue(sl):
            # psum = 0.25*<xn_i,xn_j> - mask.  dist^2 = 2 - 8*psum_dot.
            # Hardest positive = row min (masked down): dist_ap^2 =
            # relu(-8*mn - 6), exact 0 when the row has no positives.
            # Hardest negative = row max: dist_an^2 = relu(-8*mx + 2) +
            # eps; mx < -0.5 only when every column is same-class.
            nc.scalar.activation(u[:, sl], mn[:, sl], Act.Relu, bias=bm6[:], scale=-8.0)
            nc.scalar.activation(dap[:, sl], u[:, sl], Act.Sqrt)
            nc.scalar.activation(v[:, sl], mx[:, sl], Act.Relu, bias=bp2[:], scale=-8.0)
            nc.scalar.activation(sv[:, sl], v[:, sl], Act.Sqrt, bias=beps[:])
            nc.vector.tensor_scalar(e[:, sl], mx[:, sl], -0.5, None, op0=Alu.is_lt)
            # dist_an = 1.0 where the row has no negatives, else sv.
            nc.vector.copy_predicated(sv[:, sl], e[:, sl], ones4[:, sl])
            nc.vector.tensor_tensor(df[:, sl], dap[:, sl], sv[:, sl], op=Alu.subtract)
            nc.vector.tensor_scalar(
                lrow[:, sl], df[:, sl], MARGIN, 0.0, op0=Alu.add, op1=Alu.max
            )

        # Main matmul with the -1 mask folded in as a third K-chunk (lhsT
        # = -identity).  Each anchor row tile is processed as two
        # independent [128,1024] PSUM halves: the lo half (opposite groups
        # 1-2) reduces while groups 3-4 are still being transposed, and
        # freeing the lo PSUM buffer early lets the next row-tile pair
        # start its matmuls.  Row max/min per half via the fused
        # evac+accum tensor_scalar; tiny TT combines merge the halves.
        mxl = stat.tile([128, RT], f32, tag="mxl")
        mxh = stat.tile([128, RT], f32, tag="mxh")
        mnl = stat.tile([128, RT], f32, tag="mnl")
        mnh = stat.tile([128, RT], f32, tag="mnh")

        def mm_half(r, h):
            pmh = psum.tile([128, 1024], f32, tag="ps")
            for nn in range(2):
                n = 2 * h + nn
                for c in range(2):
                    nc.tensor.matmul(
                        pmh[:, ts(nn, 512)],
                        lhsT=xT[0][:, c * RPC + r * 128 : c * RPC + (r + 1) * 128],
                        rhs=xT[1 + n][:, ts(c, 512)],
                        start=(c == 0),
                        stop=False,
                    )
                nc.tensor.matmul(
                    pmh[:, ts(nn, 512)],
                    lhsT=jneg[:],
                    rhs=mq[r][:, ts(n, 512)],
                    start=False,
                    stop=True,
                )
            w16 = tree.tile([128, 1024], f16, tag="w16")
            mxt = mxl if h == 0 else mxh
            if (h == 0 and r >= 2) or (h == 1 and r < 2):
                # ACT idles once the group evacs are done: let it evacuate
                # the later lo halves so DVE only runs 4x accum-reduces.
                # The first pair stays on the fused DVE path, which fills
                # the DVE bubble between the prologue and the hi halves.
                nc.scalar.copy(w16[:], pmh[:])
                dx = tree.tile([128, 1024], f16, tag="dx")
                nc.vector.tensor_scalar(
                    dx[:], w16[:], 1.0, None,
                    op0=Alu.mult, op1=Alu.max, accum_out=mxt[:, r : r + 1],
                )
            else:
                nc.vector.tensor_scalar(
                    w16[:], pmh[:], 1.0, None,
                    op0=Alu.mult, op1=Alu.max, accum_out=mxt[:, r : r + 1],
                )
            dn = tree.tile([128, 1024], f16, tag="dn")
            nc.vector.tensor_scalar(
                dn[:], w16[:], 1.0, None,
                op0=Alu.mult, op1=Alu.min,
                accum_out=(mnl if h == 0 else mnh)[:, r : r + 1],
            )

        for pair in ((0, 1), (2, 3)):
            for r in pair:
                mm_half(r, 0)
            for r in pair:
                mm_half(r, 1)
            sl = slice(pair[0], pair[1] + 1)
            nc.vector.tensor_tensor(mx[:, sl], mxl[:, sl], mxh[:, sl], op=Alu.max)
            nc.vector.tensor_tensor(mn[:, sl], mnl[:, sl], mnh[:, sl], op=Alu.min)
            epilogue(sl)

        # Row-sum across partitions via ones-matmul, then across row tiles.
        ps2 = psum.tile([1, RT], f32, tag="ps")
        nc.tensor.matmul(ps2[:], lhsT=ones[:], rhs=lrow[:], start=True, stop=True)
        tot = stat.tile([1, 1], f32, tag="tot")
        nc.vector.tensor_reduce(tot[:], ps2[:], axis=AxX, op=Alu.add)
        nc.sync.dma_start(out[:], tot[:])

    nc.compile()
    return nc


def _get_nc():
    if "nc" not in _CACHE:
        _CACHE["nc"] = _build()
    return _CACHE["nc"]


def make_in_maps(inputs: np.ndarray, targets: np.ndarray):
    x16 = np.ascontiguousarray(inputs, dtype=np.float32).astype(np.float16)
    tf = targets.astype(np.float32)
    t16 = targets.astype(np.float16)
    in_maps = []
    for r in range(NCORES):
        rows = slice(r * RPC, (r + 1) * RPC)
        opp = slice(HALF, N) if r * RPC < HALF else slice(0, HALF)
        # xo partition k holds rows 16k..16k+15; group n covers tile slices
        # 4n..4n+3, so distance column n*512 + i*128 + k is xo-row 16k + 4n
        # + i: permute targets to match, then pre-broadcast to all
        # partitions (replaces the on-chip GPSIMD broadcast with DMA).
        to_row = (
            t16[opp].reshape(128, 4, 4).transpose(1, 2, 0).reshape(1, HALF)
        )
        in_maps.append(
            {
                # partition p holds rows 4p..4p+3 (contiguous 2KB DMA);
                # "tile" t within a group is row 4p+t.
                "xb": x16[rows].reshape(128, RT * D),
                "xo": x16[opp].reshape(128, OT * D),
                "tb": tf[rows].reshape(128, RT),
                "to": np.ascontiguousarray(np.broadcast_to(to_row, (128, HALF))),
            }
        )
    return in_maps


def kernel(inputs: np.ndarray, targets: np.ndarray) -> np.ndarray:
    from concourse.bass_utils import run_bass_kernel_spmd

    nc = _get_nc()
    in_maps = make_in_maps(inputs, targets)
    res = run_bass_kernel_spmd(nc, in_maps, list(range(NCORES)))
    total = sum(float(res.results[i]["out"][0, 0]) for i in range(NCORES))
    return np.float32(total / N)
